# revision 37
# baseline (speedup 1.0000x reference)
"""Trainium2 Bass kernel for nn_Attention_91293824844283.

Multi-head attention (identity rep): per-head 1x1-conv Q/K/V projections,
softmax(Q K^T / sqrt(E)) V, per-head output projection summed over heads.

Shapes: B=4, N=2048, D=512, H=8, E=64.

Sharding over 8 cores: core c -> (batch b = c//2, head-group g = c%2 of 4
heads). Each core computes the partial output sum over its 4 heads for its
batch; host adds the two partials per batch.

Device-side pipeline (per core), ScalarE(exp)-paced at ~1.34us per
[128,1024] tile (128 tiles of exp is the hard floor; all PE work hides
under it):
  - Flat 128-step stream over (pair, quarter, nk-tile). Step i emits
    S(i) + exp(i); PV runs SKEW=8 steps behind, buffered in SBUF pt
    tiles. The skew absorbs the projection-heavy start (all of V and K
    projections are structurally pinned inside the first quarter) and
    decouples PV stalls from the S->exp critical chain.
  - S^T per head-pair: two K=64 matmuls in disjoint PE row groups share
    one [128,1024] PSUM tile; one ACT exp per tile. V [N, 4 slots of 66]
    with a ones column per head so PV also produces the softmax
    denominators (M=65).
  - Normalization without PE transposes: rep PSUM is drained to SBUF
    immediately (frees the banks); a deferred chain does d-row K=1
    broadcast matmul -> reciprocal_approx_fast -> scale. Head s=1 of
    each pair is scaled on GPSIMD writing partitions 64:128, building a
    pair-stacked rep so the output projection contracts K=128.
  - Output projection: 2 K=128 matmuls accumulate out[nq,D] in PSUM,
    cast bf16, DMA. Host sums the two half-head partials in f32.
  - DMA: sync + scalar HWDGE queues for head-critical loads (scalar goes
    exp-only after ~13us), xv/wv on the GPSIMD SWDGE queue in parallel.
"""

import numpy as np
import ml_dtypes
from contextlib import ExitStack

B, N, D, H, E = 4, 2048, 512, 8, 64
HPC = 4            # heads per core
N_CORES = 8
NKT = N // 128     # 16 nk tiles
VSLOT = 66         # V slot: 64 V cols + 1 ones col + 1 pad
KT = D // 128      # 4 contraction tiles for projections
QW = 512           # nq quarter width
SKEW = 6           # PV lag (in tiles) behind the S/exp stream

_CACHE = {}


def _build():
    import concourse.tile as tile
    from concourse import bacc, mybir

    bf16 = mybir.dt.bfloat16
    f32 = mybir.dt.float32
    Exp = mybir.ActivationFunctionType.Exp

    nc = bacc.Bacc(
        "TRN2", target_bir_lowering=False, debug=False, num_devices=N_CORES
    )
    xqT = nc.dram_tensor("xqT", [KT, 128, N], bf16, kind="ExternalInput").ap()
    xkT = nc.dram_tensor("xkT", [KT, 128, N], bf16, kind="ExternalInput").ap()
    vT = nc.dram_tensor("vT", [KT, 128, N], bf16, kind="ExternalInput").ap()
    wqT = nc.dram_tensor("wqT", [2, 128, KT * 128], bf16, kind="ExternalInput").ap()
    wkT = nc.dram_tensor("wkT", [2, 128, KT * 128], bf16, kind="ExternalInput").ap()
    wvT = nc.dram_tensor("wvT", [128, KT * HPC * E], bf16, kind="ExternalInput").ap()
    woT = nc.dram_tensor("woT", [2, 128, D], bf16, kind="ExternalInput").ap()
    outp = nc.dram_tensor("outp", [NKT, 128, D], bf16, kind="ExternalOutput").ap()

    with tile.TileContext(nc) as tc, ExitStack() as ctx:
        cp = ctx.enter_context(tc.tile_pool(name="const", bufs=1))

        # --- persistent SBUF tiles ---
        xq = [cp.tile([128, N], bf16, tag=f"xq{k}", name=f"xq{k}") for k in range(KT)]
        xk = [cp.tile([128, N], bf16, tag=f"xk{k}", name=f"xk{k}") for k in range(KT)]
        xv = [cp.tile([128, N], bf16, tag=f"xv{k}", name=f"xv{k}") for k in range(KT)]
        wq = [cp.tile([128, KT * 128], bf16, tag=f"wq{p}", name=f"wq{p}")
              for p in range(2)]
        wk = [cp.tile([128, KT * 128], bf16, tag=f"wk{p}", name=f"wk{p}")
              for p in range(2)]
        wv = cp.tile([128, KT * HPC * E], bf16, tag="wv", name="wv")
        wost = [cp.tile([128, D], bf16, tag=f"wo{p}", name=f"wo{p}")
                for p in range(2)]
        qt = [cp.tile([128, N], bf16, tag=f"qt{p}", name=f"qt{p}") for p in range(2)]
        kt = [cp.tile([128, N], bf16, tag=f"kt{p}", name=f"kt{p}") for p in range(2)]
        vaug = [cp.tile([128, HPC, VSLOT], bf16, tag=f"va{t}", name=f"va{t}")
                for t in range(NKT)]
        repst = [cp.tile([128, N], bf16, tag=f"rs{p}", name=f"rs{p}")
                 for p in range(2)]
        onesb = cp.tile([65, E], bf16, tag="onesb")

        warm_sb = cp.tile([128, 512], bf16, tag="warm_sb")

        # --- input DMAs, deadline-ordered across three queues. All gpsimd
        # memsets ride ahead of / between the SWDGE DMAs so nothing on the
        # PE side ever waits behind a descriptor-gen burst.
        nc.gpsimd.memset(warm_sb[:], 0.0)
        nc.gpsimd.memset(onesb[:], 1.0)
        for t in range(4):
            nc.gpsimd.memset(vaug[t][:], 1.0)
        c0 = slice(0, 512)
        nc.sync.dma_start(wk[0][:], wkT[0])
        nc.scalar.dma_start(wq[0][:], wqT[0])
        for k in range(KT):          # head-critical
            nc.sync.dma_start(xk[k][:, c0], xkT[k][:, c0])
            nc.scalar.dma_start(xq[k][:, c0], xqT[k][:, c0])
        nc.gpsimd.dma_start(wv[:], wvT)   # V path on the SWDGE queue
        for c in range(4):
            sl = slice(c * 512, (c + 1) * 512)
            for k in range(KT):
                nc.gpsimd.dma_start(xv[k][:, sl], vT[k][:, sl])
            if c < 3:
                for t in range(4 * (c + 1), 4 * (c + 2)):
                    nc.gpsimd.memset(vaug[t][:], 1.0)
        for c in (1, 2):
            sl = slice(c * 512, (c + 1) * 512)
            for k in range(KT):
                nc.sync.dma_start(xk[k][:, sl], xkT[k][:, sl])
        sl = slice(512, 1024)
        for k in range(KT):
            nc.sync.dma_start(xq[k][:, sl], xqT[k][:, sl])
        sl = slice(3 * 512, 4 * 512)
        for k in range(KT):
            nc.sync.dma_start(xk[k][:, sl], xkT[k][:, sl])
        nc.sync.dma_start(wk[1][:], wkT[1])
        nc.sync.dma_start(wq[1][:], wqT[1])
        for c in (2, 3):
            sl = slice(c * 512, (c + 1) * 512)
            for k in range(KT):
                nc.sync.dma_start(xq[k][:, sl], xqT[k][:, sl])
        for p in range(2):
            nc.sync.dma_start(wost[p][:], woT[p])

        # --- PSUM pools: spair 2 banks x 2 bufs + rep 2 x 1 bank + fill
        # 2 x 1 bank = 8 banks.
        sp = ctx.enter_context(tc.tile_pool(name="spsum", bufs=2, space="PSUM"))
        rp = ctx.enter_context(tc.tile_pool(name="rpsum", bufs=1, space="PSUM"))
        fpp = ctx.enter_context(tc.tile_pool(name="fill", bufs=2, space="PSUM"))
        ptp = ctx.enter_context(tc.tile_pool(name="ptile", bufs=SKEW + 3))
        smp = ctx.enter_context(tc.tile_pool(name="small", bufs=2))

        def proj_chunk(dst, w, x, c):
            ps = fpp.tile([128, 512], f32, tag="f", name="proj_ps")
            sl = slice(c * 512, (c + 1) * 512)
            for k in range(KT):
                nc.tensor.matmul(
                    ps[:], w[:, k * 128:(k + 1) * 128], x[k][:, sl],
                    start=(k == 0), stop=(k == KT - 1),
                )
            nc.vector.tensor_copy(dst[:, sl], ps[:])

        def vproj_tile(t):
            ps = fpp.tile([128, HPC * E], f32, tag="f", name="vproj_ps")
            tsl = slice(t * 128, (t + 1) * 128)
            for k in range(KT):
                nc.tensor.matmul(
                    ps[:], xv[k][:, tsl], wv[:, k * HPC * E:(k + 1) * HPC * E],
                    start=(k == 0), stop=(k == KT - 1),
                )
            nc.vector.tensor_copy(vaug[t][:, :, 0:E], ps[:])

        # --- PE warmup burst BEFORE the head projections: spans the DMA
        # window so HAM is at K=8/8 when the stream starts. Lives in the
        # spair pool slots, NOT the fill pool: the projections must not
        # queue behind warmup via a pool-slot WAW, and S(0) is not
        # data-ready before the warmup ends so its slot wait is free.
        for i in range(6):
            wpt = sp.tile([128, 2 * QW], f32, tag="s", name="warm_ps")
            nc.tensor.matmul(wpt[:, 0:512], warm_sb[:, 0:128], warm_sb[:],
                             start=True, stop=True)

        # --- minimal head: just enough projection for the stream start.
        proj_chunk(kt[0], wk[0], xk, 0)
        proj_chunk(qt[0], wq[0], xq, 0)

        def norm_chain(k8, rrawb, split=1):
            # d (row 64 of rep~) -> K=1 broadcast matmul over 64 partitions
            # -> fast reciprocal -> scale into the pair-stacked rep. s=0 on
            # DVE (partitions 0:64), s=1 on GPSIMD writing 64:128. split>1
            # chops the scales into column chunks so a consumer of the first
            # chunk (tail outproj) is not gated by the full-width op.
            p, q4 = k8 // 4, k8 % 4
            qoff = q4 * QW
            cw = QW // split
            for s in range(2):
                dbp = fpp.tile([E, QW], f32, tag="f", name="dbp")
                nc.tensor.matmul(dbp[:], onesb[64:65, :], rrawb[s][64:65, :],
                                 start=True, stop=True)
                dinvb = smp.tile([E, QW], f32, tag=f"dv{s}", name="dinvb")
                with nc.allow_low_precision(reason="softmax denom reciprocal"):
                    nc.vector.reciprocal_approx_fast(dinvb[:], dbp[:])
                    for c in range(split):
                        csl = slice(c * cw, (c + 1) * cw)
                        osl = slice(qoff + c * cw, qoff + (c + 1) * cw)
                        if s == 0:
                            nc.vector.tensor_mul(
                                repst[p][0:E, osl], rrawb[s][0:E, csl],
                                dinvb[:, csl])
                        else:
                            nc.gpsimd.tensor_mul(
                                repst[p][E:128, osl], rrawb[s][0:E, csl],
                                dinvb[:, csl])

        def outproj_tile(t, tail=False):
            # tail tiles run after the last exp: ScalarE and its HWDGE
            # queue are idle, so alternate cast/DMA across both engines
            # to halve the serial tail.
            tsl = slice(t * 128, (t + 1) * 128)
            ops = fpp.tile([128, D], f32, tag="f", name="ops")
            for p in range(2):
                nc.tensor.matmul(
                    ops[:], repst[p][:, tsl], wost[p][:],
                    start=(p == 0), stop=(p == 1),
                )
            ost = ptp.tile([128, D], bf16, tag="ost")
            if tail and t % 2:
                nc.scalar.activation(ost[:], ops[:],
                                     mybir.ActivationFunctionType.Copy)
                nc.scalar.dma_start(outp[t], ost[:])
            else:
                nc.vector.tensor_copy(ost[:], ops[:])
                nc.sync.dma_start(outp[t], ost[:])

        # --- flat skewed stream ---------------------------------------
        NSTEP = 2 * 4 * NKT          # 128
        rep_ps = {}                  # quarter-index -> [rep_ps tiles]
        rrawb = {}                   # quarter-index -> [sbuf drains]
        pts = {}                     # step -> pt tile

        hooks = {}

        def add_hook(i, fn):
            hooks.setdefault(i, []).append(fn)

        # projections (consumer step -> emit 2 steps early)
        add_hook(3, lambda: proj_chunk(kt[0], wk[0], xk, 1))
        add_hook(7, lambda: proj_chunk(kt[0], wk[0], xk, 2))
        add_hook(10, lambda: proj_chunk(kt[0], wk[0], xk, 3))
        add_hook(14, lambda: proj_chunk(qt[0], wq[0], xq, 1))
        add_hook(30, lambda: proj_chunk(qt[0], wq[0], xq, 2))
        add_hook(46, lambda: proj_chunk(qt[0], wq[0], xq, 3))
        # pair-1 projections pushed into the lightest quarters; deadlines:
        # kt[1] chunk c by step 64+4c, qt[1] chunk c by step 64+16c.
        # Each is split across two consecutive hooks so the in-order PE
        # queue never takes more than a half-chunk bite per stream step.
        def add_split_proj(h, dst, w, x, c):
            box = []

            def half1():
                ps = fpp.tile([128, 512], f32, tag="f", name="proj_ps")
                box.append(ps)
                sl = slice(c * 512, (c + 1) * 512)
                for k in (0, 1):
                    nc.tensor.matmul(
                        ps[:], w[:, k * 128:(k + 1) * 128], x[k][:, sl],
                        start=(k == 0), stop=False,
                    )

            def half2():
                ps = box.pop()
                sl = slice(c * 512, (c + 1) * 512)
                for k in (2, 3):
                    nc.tensor.matmul(
                        ps[:], w[:, k * 128:(k + 1) * 128], x[k][:, sl],
                        start=False, stop=(k == 3),
                    )
                nc.vector.tensor_copy(dst[:, sl], ps[:])

            add_hook(h, half1)
            add_hook(h + 1, half2)

        add_split_proj(36, kt[1], wk[1], xk, 0)
        add_split_proj(44, kt[1], wk[1], xk, 1)
        add_split_proj(52, kt[1], wk[1], xk, 2)
        add_split_proj(58, kt[1], wk[1], xk, 3)
        add_split_proj(61, qt[1], wq[1], xq, 0)
        add_split_proj(76, qt[1], wq[1], xq, 1)
        add_split_proj(92, qt[1], wq[1], xq, 2)
        add_split_proj(108, qt[1], wq[1], xq, 3)
        # vproj(t) just before PV(0,0,t) at step t+SKEW
        for t in range(NKT):
            add_hook(max(0, t + SKEW - 1), lambda t=t: vproj_tile(t))
        # normalize chain for quarter k8 (PV done at step 16*k8+15+SKEW;
        # +4 gives the previous outproj cast time to free the fill slot
        # so the bcast matmul cannot stall the in-order PE queue)
        for k8 in range(7):
            add_hook(16 * k8 + 15 + SKEW + 4,
                     lambda k8=k8: norm_chain(k8, rrawb.pop(k8)))
        # output projection: quarter k8 (pair1 q = k8-4) tiles 4q..4q+3,
        # packed right after the chain so nothing spills past stream end
        for k8 in range(4, 7):
            for j in range(4):
                add_hook(16 * k8 + 15 + SKEW + 6 + j,
                         lambda k8=k8, j=j: outproj_tile(4 * (k8 - 4) + j))

        spair = {}
        for i in range(NSTEP + SKEW):
            if i < NSTEP:
                p, q4, t = i // 64, (i // 16) % 4, i % 16
                k8 = i // 16
                if t == 0:
                    rep_ps[k8] = [
                        rp.tile([65, QW], f32, tag=f"rep{s}", name=f"rep{s}")
                        for s in range(2)
                    ]
                tsl = slice(t * 128, (t + 1) * 128)
                qoff = q4 * QW
                sb = sp.tile([128, 2 * QW], f32, tag="s", name="spair")
                spair[i] = sb
                for s in range(2):
                    esl = slice(s * 64, (s + 1) * 64)
                    nc.tensor.matmul(
                        sb[:, s * QW:(s + 1) * QW],
                        kt[p][esl, tsl], qt[p][esl, qoff:qoff + QW],
                        start=True, stop=True,
                    )
                ptt = ptp.tile([128, 2 * QW], bf16, tag="p", name="pt")
                nc.scalar.activation(ptt[:], sb[:], Exp)
                pts[i] = ptt
                del spair[i]
            j = i - SKEW
            if 0 <= j < NSTEP:
                p, q4, t = j // 64, (j // 16) % 4, j % 16
                k8 = j // 16
                ptt = pts.pop(j)
                for s in range(2):
                    h = 2 * p + s
                    nc.tensor.matmul(
                        rep_ps[k8][s][:],
                        vaug[t][:, h, 0:65], ptt[:, s * QW:(s + 1) * QW],
                        start=(t == 0), stop=(t == NKT - 1),
                    )
                if t == NKT - 1:
                    rr = []
                    for s in range(2):
                        r = smp.tile([65, QW], bf16, tag=f"rr{s}", name=f"rr{s}")
                        nc.vector.tensor_copy(r[:], rep_ps[k8][s][:])
                        rr.append(r)
                    rrawb[k8] = rr
                    del rep_ps[k8]
            for fn in hooks.get(i, ()):
                fn()
        # --- tail: last quarter normalize (per-tile chunks) + final tiles
        norm_chain(7, rrawb.pop(7), split=4)
        for t in range(12, 16):
            outproj_tile(t, tail=True)

    nc.compile()
    return nc


def _prep_core_inputs(c, x1, x2, v, Wq, Wk, Wv, Wo, identf=None, identb=None):
    bf = ml_dtypes.bfloat16
    b, g = c // 2, c % 2
    hs = slice(g * HPC, (g + 1) * HPC)
    wq = (Wq[hs] * (1.0 / np.sqrt(E))).astype(np.float32)   # fold 1/sqrt(E)
    wk, wv, wo = Wk[hs], Wv[hs], Wo[hs]

    def t_pack_pair(w):
        # [4,E,D] -> per pair p: concat(w[2p].T, w[2p+1].T, axis=1) [D,128]
        # packed as one [128, KT*128] tile: [p_row, k*128+e] = m[k*128+p, e]
        out = np.empty((2, 128, KT * 128), bf)
        for p in range(2):
            m = np.concatenate([w[2 * p].T, w[2 * p + 1].T], axis=1)  # [D,128]
            out[p] = (
                m.reshape(KT, 128, 128).transpose(1, 0, 2).reshape(128, KT * 128)
            ).astype(bf)
        return out

    xq = np.ascontiguousarray(x2[b].T).astype(bf).reshape(KT, 128, N)
    xk = np.ascontiguousarray(x1[b].T).astype(bf).reshape(KT, 128, N)
    xv = np.ascontiguousarray(v[b].T).astype(bf).reshape(KT, 128, N)
    wvT = np.concatenate([wv[h].T for h in range(HPC)], axis=1)  # [D, 256]
    wvP = (
        wvT.reshape(KT, 128, HPC * E).transpose(1, 0, 2).reshape(128, KT * HPC * E)
    )
    # pair-stacked output weights: [2, 128, D], rows = [E of h=2p; E of 2p+1]
    woT = np.stack([
        np.concatenate([wo[2 * p].T, wo[2 * p + 1].T], axis=0)
        for p in range(2)
    ])
    return {
        "xqT": xq, "xkT": xk, "vT": xv,
        "wqT": t_pack_pair(wq), "wkT": t_pack_pair(wk),
        "wvT": np.ascontiguousarray(wvP).astype(bf),
        "woT": woT.astype(bf),
    }


def kernel(**inputs):
    from concourse.bass_utils import run_bass_kernel_spmd

    x1 = np.asarray(inputs["x1"], np.float32)
    x2 = np.asarray(inputs["x2"], np.float32)
    v = np.asarray(inputs["v"], np.float32)
    Wq = np.asarray(inputs["Wq"], np.float32)
    Wk = np.asarray(inputs["Wk"], np.float32)
    Wv = np.asarray(inputs["Wv"], np.float32)
    Wo = np.asarray(inputs["Wo"], np.float32)

    if "nc" not in _CACHE:
        _CACHE["nc"] = _build()
    nc = _CACHE["nc"]

    in_maps = [
        _prep_core_inputs(c, x1, x2, v, Wq, Wk, Wv, Wo)
        for c in range(N_CORES)
    ]
    res = run_bass_kernel_spmd(nc, in_maps, list(range(N_CORES)))
    out = np.empty((B, N, D), np.float32)
    for b in range(B):
        out[b] = (
            res.results[2 * b]["outp"].reshape(N, D).astype(np.float32)
            + res.results[2 * b + 1]["outp"].reshape(N, D).astype(np.float32)
        )
    return out
